# revision 4
# baseline (speedup 1.0000x reference)
"""Trainium2 Bass kernel for nn_LstmClassifier.

Reference computation (B=2048, IN=1024, HID=1024, T=32, OUT=512):
    h0 = relu(x @ W_in.T + b_in);  c0 = 0
    repeat T times:  gates = h @ W_ih.T + b_ih + h @ W_hh.T + b_hh
                     i,f,g,o = split(gates); c = sig(f)*c + sig(i)*tanh(g)
                     h = sig(o)*tanh(c);     collect h
    out[:, t, :] = h_t @ W_out.T + b_out

Key algebraic fusion: the cell input is the previous hidden state, so
    gates = h @ (W_ih + W_hh).T + (b_ih + b_hh)
one matmul per step instead of two.

Distribution: pure data-parallel over the batch dim, 256 rows per core on
8 NeuronCores, weights replicated, no collectives.

On-chip layout is "H-major": the hidden state lives transposed as
hT[hid=1024 part-dim over 8 tiles of 128, batch=256 free-dim].  The
per-step matmul is then gatesT[g,b] = sum_k WT[k,g] * hT[k,b] with the
(fused, transposed) weight as the stationary operand and hT as the moving
operand -- the recurrence needs no transposes anywhere, and the gate bias
is constant along the free dim, i.e. a per-partition scalar that the
scalar engine's activation instruction adds for free.

Matmul dtype: float32r (fp32 stored in SBUF, PE truncates to FP22 -- 1.5
cycles/row vs 2.0 for exact fp32, ~6e-5 per-element rounding).
"""

import os
import sys

sys.path.insert(0, "/opt/trn_rl_repo")

import numpy as np

B, IN_DIM, HID, T, OUT_DIM = 2048, 1024, 1024, 32, 512
N_CORES = 8
BS = B // N_CORES  # batch rows per core
KT = HID // 128  # contraction tiles (8)
GT = 4 * HID // 128  # gate tiles (32)
MT_OUT = OUT_DIM // 128  # out-proj tiles (4)

# "fp16" (default) | "bf16" | "fp32r"
MM_MODE = os.environ.get("BASS_LSTM_MMDT", "fp16")

_cache = {}


def _np_w_dtype():
    if MM_MODE == "bf16":
        import ml_dtypes

        return ml_dtypes.bfloat16
    if MM_MODE == "fp16":
        return np.float16
    return np.float32


def _build():
    import concourse.bacc as bacc
    import concourse.mybir as mybir
    from concourse.tile import TileContext

    f32 = mybir.dt.float32
    if MM_MODE == "bf16":
        dt_w = mybir.dt.bfloat16
    elif MM_MODE == "fp16":
        dt_w = mybir.dt.float16
    else:
        dt_w = f32
    AF = mybir.ActivationFunctionType

    def mm_ap(ap):
        # fp32r is layout-identical to fp32; flip the dtype right at the
        # matmul operand so PE runs the 1.5-cycle FP22 path.
        if MM_MODE == "fp32r":
            return ap.bitcast(mybir.dt.float32r)
        return ap

    nc = bacc.Bacc(
        "TRN2", target_bir_lowering=False, debug=False, num_devices=N_CORES
    )

    xT = nc.dram_tensor("xT", [IN_DIM, BS], dt_w, kind="ExternalInput")
    wg = nc.dram_tensor("wg", [HID, 4 * HID], dt_w, kind="ExternalInput")
    wi = nc.dram_tensor("wi", [IN_DIM, HID], dt_w, kind="ExternalInput")
    wo = nc.dram_tensor("wo", [HID, OUT_DIM], dt_w, kind="ExternalInput")
    bg = nc.dram_tensor("bg", [128, GT], f32, kind="ExternalInput")
    bi = nc.dram_tensor("bi", [128, KT], f32, kind="ExternalInput")
    bo = nc.dram_tensor("bo", [128, MT_OUT], f32, kind="ExternalInput")
    outT = nc.dram_tensor("outT", [T, OUT_DIM, BS], f32, kind="ExternalOutput")

    with TileContext(nc) as tc:
        with (
            tc.tile_pool(name="wgp", bufs=1) as wg_pool,
            tc.tile_pool(name="wop", bufs=1) as wo_pool,
            tc.tile_pool(name="state", bufs=1) as st_pool,
            tc.tile_pool(name="biasp", bufs=1) as b_pool,
            tc.tile_pool(name="gpsum", bufs=6, space="PSUM") as gp_pool,
            tc.tile_pool(name="opsum", bufs=2, space="PSUM") as op_pool,
        ):
            wg_sb = []
            for k in range(KT):
                wt = wg_pool.tile([128, 4 * HID], dt_w, tag=f"wg{k}", name=f"wg{k}")
                nc.sync.dma_start(out=wt[:], in_=wg[k * 128 : (k + 1) * 128, :])
                wg_sb.append(wt)
            wo_sb = []
            for k in range(KT):
                wt = wo_pool.tile([128, OUT_DIM], dt_w, tag=f"wo{k}", name=f"wo{k}")
                nc.sync.dma_start(out=wt[:], in_=wo[k * 128 : (k + 1) * 128, :])
                wo_sb.append(wt)

            bg_sb = b_pool.tile([128, GT], f32, tag="bg", name="bg_sb")
            nc.sync.dma_start(out=bg_sb[:], in_=bg[:, :])
            bi_sb = b_pool.tile([128, KT], f32, tag="bi", name="bi_sb")
            nc.sync.dma_start(out=bi_sb[:], in_=bi[:, :])
            bo_sb = b_pool.tile([128, MT_OUT], f32, tag="bo", name="bo_sb")
            nc.sync.dma_start(out=bo_sb[:], in_=bo[:, :])

            # double-buffered hidden state (H-major), persistent cell state
            h_tiles = [
                [
                    st_pool.tile([128, BS], dt_w, tag=f"h{p}_{j}", name=f"h{p}_{j}")
                    for j in range(KT)
                ]
                for p in range(2)
            ]
            c_tiles = [
                st_pool.tile([128, BS], f32, tag=f"c{j}", name=f"c{j}")
                for j in range(KT)
            ]
            for j in range(KT):
                nc.vector.memset(c_tiles[j][:], 0.0)

            # ---- input projection: h0 = relu(W_in @ x.T + b_in), H-major
            with (
                tc.tile_pool(name="xp", bufs=1) as x_pool,
                tc.tile_pool(name="winp", bufs=16) as wi_pool,
            ):
                xT_sb = []
                for k in range(KT):
                    xt = x_pool.tile([128, BS], dt_w, tag=f"x{k}", name=f"x{k}")
                    nc.sync.dma_start(out=xt[:], in_=xT[k * 128 : (k + 1) * 128, :])
                    xT_sb.append(xt)
                for m in range(KT):
                    ps = gp_pool.tile([128, BS], f32, tag="gp", name=f"ip_ps{m}")
                    for k in range(KT):
                        wt = wi_pool.tile([128, 128], dt_w, tag="win", name=f"wi{k}_{m}")
                        nc.sync.dma_start(
                            out=wt[:],
                            in_=wi[k * 128 : (k + 1) * 128, m * 128 : (m + 1) * 128],
                        )
                        nc.tensor.matmul(
                            ps[:],
                            lhsT=mm_ap(wt[:]),
                            rhs=mm_ap(xT_sb[k][:]),
                            start=(k == 0),
                            stop=(k == KT - 1),
                        )
                    nc.scalar.activation(
                        h_tiles[0][m][:], ps[:], AF.Relu, bias=bi_sb[:, m : m + 1]
                    )

            # ---- recurrence
            GATE_FUNCS = [AF.Sigmoid, AF.Sigmoid, AF.Tanh, AF.Sigmoid]  # i f g o
            with (
                tc.tile_pool(name="gact", bufs=12) as ga_pool,
                tc.tile_pool(name="osb", bufs=4) as os_pool,
            ):

                def out_proj(hsrc, t_idx):
                    # outT[t_idx] = W_out @ h + b_out  (H-major h, [OUT,BS] out)
                    for m in range(MT_OUT):
                        ps = op_pool.tile([128, BS], f32, tag="op", name=f"op{t_idx}_{m}")
                        for k in range(KT):
                            nc.tensor.matmul(
                                ps[:],
                                lhsT=mm_ap(wo_sb[k][:, m * 128 : (m + 1) * 128]),
                                rhs=mm_ap(hsrc[k][:]),
                                start=(k == 0),
                                stop=(k == KT - 1),
                            )
                        ot = os_pool.tile([128, BS], f32, tag="ot", name=f"ot{t_idx}_{m}")
                        nc.scalar.activation(
                            ot[:], ps[:], AF.Identity, bias=bo_sb[:, m : m + 1]
                        )
                        nc.sync.dma_start(
                            out=outT[t_idx, m * 128 : (m + 1) * 128, :], in_=ot[:]
                        )

                for t in range(T):
                    hp = h_tiles[t % 2]
                    hn = h_tiles[(t + 1) % 2]
                    for j in range(KT):
                        acts = []
                        for gi in range(4):
                            g_idx = gi * KT + j
                            ps = gp_pool.tile(
                                [128, BS], f32, tag="gp", name=f"ps{t}_{j}_{gi}"
                            )
                            for k in range(KT):
                                nc.tensor.matmul(
                                    ps[:],
                                    lhsT=mm_ap(
                                        wg_sb[k][:, g_idx * 128 : (g_idx + 1) * 128]
                                    ),
                                    rhs=mm_ap(hp[k][:]),
                                    start=(k == 0),
                                    stop=(k == KT - 1),
                                )
                            at = ga_pool.tile(
                                [128, BS], f32, tag="ga", name=f"a{t}_{j}_{gi}"
                            )
                            nc.scalar.activation(
                                at[:],
                                ps[:],
                                GATE_FUNCS[gi],
                                bias=bg_sb[:, g_idx : g_idx + 1],
                            )
                            acts.append(at)
                        a_i, a_f, a_g, a_o = acts
                        fc = ga_pool.tile([128, BS], f32, tag="ga", name=f"fc{t}_{j}")
                        nc.vector.tensor_mul(fc[:], a_f[:], c_tiles[j][:])
                        ig = ga_pool.tile([128, BS], f32, tag="ga", name=f"ig{t}_{j}")
                        nc.vector.tensor_mul(ig[:], a_i[:], a_g[:])
                        nc.vector.tensor_add(c_tiles[j][:], fc[:], ig[:])
                        th = ga_pool.tile([128, BS], f32, tag="ga", name=f"th{t}_{j}")
                        nc.scalar.activation(th[:], c_tiles[j][:], AF.Tanh)
                        nc.vector.tensor_mul(hn[j][:], a_o[:], th[:])
                    if t >= 1:
                        # project the PREVIOUS step's h: its inputs are ready,
                        # so these matmuls keep PE busy while the ACT/DVE
                        # epilogue of this step finishes h_{t+1}.
                        out_proj(hp, t - 1)
                out_proj(h_tiles[T % 2], T - 1)

    nc.finalize()
    return nc


def _get_nc():
    if "nc" not in _cache:
        _cache["nc"] = _build()
    return _cache["nc"]


def _prep_in_maps(x, W_in, b_in, W_ih, b_ih, W_hh, b_hh, W_out, b_out):
    wdt = _np_w_dtype()

    f32 = np.float32
    wg_np = np.ascontiguousarray((W_ih + W_hh).T.astype(wdt))
    wi_np = np.ascontiguousarray(W_in.T.astype(wdt))
    wo_np = np.ascontiguousarray(W_out.T.astype(wdt))
    bg_np = np.ascontiguousarray((b_ih + b_hh).astype(f32).reshape(GT, 128).T)
    bi_np = np.ascontiguousarray(b_in.astype(f32).reshape(KT, 128).T)
    bo_np = np.ascontiguousarray(b_out.astype(f32).reshape(MT_OUT, 128).T)

    in_maps = []
    for c in range(N_CORES):
        xT_c = np.ascontiguousarray(x[c * BS : (c + 1) * BS].T.astype(wdt))
        in_maps.append(
            {
                "xT": xT_c,
                "wg": wg_np,
                "wi": wi_np,
                "wo": wo_np,
                "bg": bg_np,
                "bi": bi_np,
                "bo": bo_np,
            }
        )

    return in_maps


def _assemble(results):
    out = np.concatenate(
        [results[c]["outT"].transpose(2, 0, 1) for c in range(N_CORES)], axis=0
    )
    return np.ascontiguousarray(out.astype(np.float32))


def kernel(x, W_in, b_in, W_ih, b_ih, W_hh, b_hh, W_out, b_out):
    from concourse.bass_utils import run_bass_kernel_spmd

    nc = _get_nc()
    in_maps = _prep_in_maps(
        x, W_in, b_in, W_ih, b_ih, W_hh, b_hh, W_out, b_out
    )
    res = run_bass_kernel_spmd(nc, in_maps, list(range(N_CORES)))
    return _assemble(res.results)


# revision 6
# speedup vs baseline: 1.0026x; 1.0026x over previous
"""Trainium2 Bass kernel for nn_LstmClassifier.

Reference computation (B=2048, IN=1024, HID=1024, T=32, OUT=512):
    h0 = relu(x @ W_in.T + b_in);  c0 = 0
    repeat T times:  gates = h @ W_ih.T + b_ih + h @ W_hh.T + b_hh
                     i,f,g,o = split(gates); c = sig(f)*c + sig(i)*tanh(g)
                     h = sig(o)*tanh(c);     collect h
    out[:, t, :] = h_t @ W_out.T + b_out

Key algebraic fusion: the cell input is the previous hidden state, so
    gates = h @ (W_ih + W_hh).T + (b_ih + b_hh)
one matmul per step instead of two.

Distribution: pure data-parallel over the batch dim, 256 rows per core on
8 NeuronCores, weights replicated, no collectives.

On-chip layout is "H-major": the hidden state lives transposed as
hT[hid=1024 part-dim over 8 tiles of 128, batch=256 free-dim].  The
per-step matmul is then gatesT[g,b] = sum_k WT[k,g] * hT[k,b] with the
(fused, transposed) weight as the stationary operand and hT as the moving
operand -- the recurrence needs no transposes anywhere, and the gate bias
is constant along the free dim, i.e. a per-partition scalar that the
scalar engine's activation instruction adds for free.

Matmul dtype: float32r (fp32 stored in SBUF, PE truncates to FP22 -- 1.5
cycles/row vs 2.0 for exact fp32, ~6e-5 per-element rounding).
"""

import os
import sys

sys.path.insert(0, "/opt/trn_rl_repo")

import numpy as np

B, IN_DIM, HID, T, OUT_DIM = 2048, 1024, 1024, 32, 512
N_CORES = 8
BS = B // N_CORES  # batch rows per core
KT = HID // 128  # contraction tiles (8)
GT = 4 * HID // 128  # gate tiles (32)
MT_OUT = OUT_DIM // 128  # out-proj tiles (4)

# "fp16" (default) | "bf16" | "fp32r"
MM_MODE = os.environ.get("BASS_LSTM_MMDT", "fp16")

_cache = {}


def _np_w_dtype():
    if MM_MODE == "bf16":
        import ml_dtypes

        return ml_dtypes.bfloat16
    if MM_MODE == "fp16":
        return np.float16
    return np.float32


def _build():
    import concourse.bacc as bacc
    import concourse.mybir as mybir
    from concourse.tile import TileContext

    f32 = mybir.dt.float32
    if MM_MODE == "bf16":
        dt_w = mybir.dt.bfloat16
    elif MM_MODE == "fp16":
        dt_w = mybir.dt.float16
    else:
        dt_w = f32
    AF = mybir.ActivationFunctionType

    def mm_ap(ap):
        # fp32r is layout-identical to fp32; flip the dtype right at the
        # matmul operand so PE runs the 1.5-cycle FP22 path.
        if MM_MODE == "fp32r":
            return ap.bitcast(mybir.dt.float32r)
        return ap

    nc = bacc.Bacc(
        "TRN2", target_bir_lowering=False, debug=False, num_devices=N_CORES
    )

    xT = nc.dram_tensor("xT", [IN_DIM, BS], dt_w, kind="ExternalInput")
    wg = nc.dram_tensor("wg", [HID, 4 * HID], dt_w, kind="ExternalInput")
    wi = nc.dram_tensor("wi", [IN_DIM, HID], dt_w, kind="ExternalInput")
    wo = nc.dram_tensor("wo", [HID, OUT_DIM], dt_w, kind="ExternalInput")
    bg = nc.dram_tensor("bg", [128, GT], f32, kind="ExternalInput")
    bi = nc.dram_tensor("bi", [128, KT], f32, kind="ExternalInput")
    bo = nc.dram_tensor("bo", [128, MT_OUT], f32, kind="ExternalInput")
    outT = nc.dram_tensor("outT", [T, OUT_DIM, BS], f32, kind="ExternalOutput")

    with TileContext(nc) as tc:
        with (
            tc.tile_pool(name="wgp", bufs=1) as wg_pool,
            tc.tile_pool(name="wop", bufs=1) as wo_pool,
            tc.tile_pool(name="state", bufs=1) as st_pool,
            tc.tile_pool(name="biasp", bufs=1) as b_pool,
            tc.tile_pool(name="gpsum", bufs=6, space="PSUM") as gp_pool,
            tc.tile_pool(name="opsum", bufs=2, space="PSUM") as op_pool,
        ):
            # tiny bias loads first (first ACT needs bi almost immediately)
            bg_sb = b_pool.tile([128, GT], f32, tag="bg", name="bg_sb")
            nc.sync.dma_start(out=bg_sb[:], in_=bg[:, :])
            bi_sb = b_pool.tile([128, KT], f32, tag="bi", name="bi_sb")
            nc.sync.dma_start(out=bi_sb[:], in_=bi[:, :])
            bo_sb = b_pool.tile([128, MT_OUT], f32, tag="bo", name="bo_sb")
            nc.sync.dma_start(out=bo_sb[:], in_=bo[:, :])

            # big recurrence weights: spread across idle engines' DMA queues
            # so they land in parallel with the input projection's loads,
            # which stay on sync.  Step 0 needs ALL of wg, so wall-clock to
            # full residency is what matters.
            wg_dma_engines = [nc.gpsimd, nc.scalar]
            wg_sb = []
            for k in range(KT):
                wt = wg_pool.tile([128, 4 * HID], dt_w, tag=f"wg{k}", name=f"wg{k}")
                eng = wg_dma_engines[k % len(wg_dma_engines)]
                half = 2 * HID
                eng.dma_start(out=wt[:, :half], in_=wg[k * 128 : (k + 1) * 128, :half])
                eng.dma_start(out=wt[:, half:], in_=wg[k * 128 : (k + 1) * 128, half:])
                wg_sb.append(wt)
            wo_sb = []
            for k in range(KT):
                wt = wo_pool.tile([128, OUT_DIM], dt_w, tag=f"wo{k}", name=f"wo{k}")
                nc.gpsimd.dma_start(out=wt[:], in_=wo[k * 128 : (k + 1) * 128, :])
                wo_sb.append(wt)

            # double-buffered hidden state (H-major), persistent cell state
            h_tiles = [
                [
                    st_pool.tile([128, BS], dt_w, tag=f"h{p}_{j}", name=f"h{p}_{j}")
                    for j in range(KT)
                ]
                for p in range(2)
            ]
            c_tiles = [
                st_pool.tile([128, BS], f32, tag=f"c{j}", name=f"c{j}")
                for j in range(KT)
            ]
            for j in range(KT):
                nc.vector.memset(c_tiles[j][:], 0.0)

            # ---- input projection: h0 = relu(W_in @ x.T + b_in), H-major
            with (
                tc.tile_pool(name="xp", bufs=1) as x_pool,
                tc.tile_pool(name="winp", bufs=16) as wi_pool,
            ):
                xT_sb = []
                for k in range(KT):
                    xt = x_pool.tile([128, BS], dt_w, tag=f"x{k}", name=f"x{k}")
                    nc.sync.dma_start(out=xt[:], in_=xT[k * 128 : (k + 1) * 128, :])
                    xT_sb.append(xt)
                for m in range(KT):
                    ps = gp_pool.tile([128, BS], f32, tag="gp", name=f"ip_ps{m}")
                    for k in range(KT):
                        wt = wi_pool.tile([128, 128], dt_w, tag="win", name=f"wi{k}_{m}")
                        nc.sync.dma_start(
                            out=wt[:],
                            in_=wi[k * 128 : (k + 1) * 128, m * 128 : (m + 1) * 128],
                        )
                        nc.tensor.matmul(
                            ps[:],
                            lhsT=mm_ap(wt[:]),
                            rhs=mm_ap(xT_sb[k][:]),
                            start=(k == 0),
                            stop=(k == KT - 1),
                        )
                    nc.scalar.activation(
                        h_tiles[0][m][:], ps[:], AF.Relu, bias=bi_sb[:, m : m + 1]
                    )

            # ---- recurrence
            GATE_FUNCS = [AF.Sigmoid, AF.Sigmoid, AF.Tanh, AF.Sigmoid]  # i f g o
            with (
                tc.tile_pool(name="gact", bufs=12) as ga_pool,
                tc.tile_pool(name="osb", bufs=4) as os_pool,
            ):

                def out_proj(hsrc, t_idx):
                    # outT[t_idx] = W_out @ h + b_out  (H-major h, [OUT,BS] out)
                    for m in range(MT_OUT):
                        ps = op_pool.tile([128, BS], f32, tag="op", name=f"op{t_idx}_{m}")
                        for k in range(KT):
                            nc.tensor.matmul(
                                ps[:],
                                lhsT=mm_ap(wo_sb[k][:, m * 128 : (m + 1) * 128]),
                                rhs=mm_ap(hsrc[k][:]),
                                start=(k == 0),
                                stop=(k == KT - 1),
                            )
                        ot = os_pool.tile([128, BS], f32, tag="ot", name=f"ot{t_idx}_{m}")
                        nc.scalar.activation(
                            ot[:], ps[:], AF.Identity, bias=bo_sb[:, m : m + 1]
                        )
                        nc.sync.dma_start(
                            out=outT[t_idx, m * 128 : (m + 1) * 128, :], in_=ot[:]
                        )

                for t in range(T):
                    hp = h_tiles[t % 2]
                    hn = h_tiles[(t + 1) % 2]
                    for j in range(KT):
                        acts = []
                        for gi in range(4):
                            g_idx = gi * KT + j
                            ps = gp_pool.tile(
                                [128, BS], f32, tag="gp", name=f"ps{t}_{j}_{gi}"
                            )
                            for k in range(KT):
                                nc.tensor.matmul(
                                    ps[:],
                                    lhsT=mm_ap(
                                        wg_sb[k][:, g_idx * 128 : (g_idx + 1) * 128]
                                    ),
                                    rhs=mm_ap(hp[k][:]),
                                    start=(k == 0),
                                    stop=(k == KT - 1),
                                )
                            at = ga_pool.tile(
                                [128, BS], f32, tag="ga", name=f"a{t}_{j}_{gi}"
                            )
                            nc.scalar.activation(
                                at[:],
                                ps[:],
                                GATE_FUNCS[gi],
                                bias=bg_sb[:, g_idx : g_idx + 1],
                            )
                            acts.append(at)
                        a_i, a_f, a_g, a_o = acts
                        fc = ga_pool.tile([128, BS], f32, tag="ga", name=f"fc{t}_{j}")
                        nc.vector.tensor_mul(fc[:], a_f[:], c_tiles[j][:])
                        ig = ga_pool.tile([128, BS], f32, tag="ga", name=f"ig{t}_{j}")
                        nc.vector.tensor_mul(ig[:], a_i[:], a_g[:])
                        nc.vector.tensor_add(c_tiles[j][:], fc[:], ig[:])
                        th = ga_pool.tile([128, BS], f32, tag="ga", name=f"th{t}_{j}")
                        nc.scalar.activation(th[:], c_tiles[j][:], AF.Tanh)
                        nc.vector.tensor_mul(hn[j][:], a_o[:], th[:])
                    if t >= 1:
                        # project the PREVIOUS step's h: its inputs are ready,
                        # so these matmuls keep PE busy while the ACT/DVE
                        # epilogue of this step finishes h_{t+1}.
                        out_proj(hp, t - 1)
                out_proj(h_tiles[T % 2], T - 1)

    nc.finalize()
    return nc


def _get_nc():
    if "nc" not in _cache:
        _cache["nc"] = _build()
    return _cache["nc"]


def _prep_in_maps(x, W_in, b_in, W_ih, b_ih, W_hh, b_hh, W_out, b_out):
    wdt = _np_w_dtype()

    f32 = np.float32
    wg_np = np.ascontiguousarray((W_ih + W_hh).T.astype(wdt))
    wi_np = np.ascontiguousarray(W_in.T.astype(wdt))
    wo_np = np.ascontiguousarray(W_out.T.astype(wdt))
    bg_np = np.ascontiguousarray((b_ih + b_hh).astype(f32).reshape(GT, 128).T)
    bi_np = np.ascontiguousarray(b_in.astype(f32).reshape(KT, 128).T)
    bo_np = np.ascontiguousarray(b_out.astype(f32).reshape(MT_OUT, 128).T)

    in_maps = []
    for c in range(N_CORES):
        xT_c = np.ascontiguousarray(x[c * BS : (c + 1) * BS].T.astype(wdt))
        in_maps.append(
            {
                "xT": xT_c,
                "wg": wg_np,
                "wi": wi_np,
                "wo": wo_np,
                "bg": bg_np,
                "bi": bi_np,
                "bo": bo_np,
            }
        )

    return in_maps


def _assemble(results):
    out = np.concatenate(
        [results[c]["outT"].transpose(2, 0, 1) for c in range(N_CORES)], axis=0
    )
    return np.ascontiguousarray(out.astype(np.float32))


def kernel(x, W_in, b_in, W_ih, b_ih, W_hh, b_hh, W_out, b_out):
    from concourse.bass_utils import run_bass_kernel_spmd

    nc = _get_nc()
    in_maps = _prep_in_maps(
        x, W_in, b_in, W_ih, b_ih, W_hh, b_hh, W_out, b_out
    )
    res = run_bass_kernel_spmd(nc, in_maps, list(range(N_CORES)))
    return _assemble(res.results)


# revision 8
# speedup vs baseline: 1.0327x; 1.0300x over previous
"""Trainium2 Bass kernel for nn_LstmClassifier.

Reference computation (B=2048, IN=1024, HID=1024, T=32, OUT=512):
    h0 = relu(x @ W_in.T + b_in);  c0 = 0
    repeat T times:  gates = h @ W_ih.T + b_ih + h @ W_hh.T + b_hh
                     i,f,g,o = split(gates); c = sig(f)*c + sig(i)*tanh(g)
                     h = sig(o)*tanh(c);     collect h
    out[:, t, :] = h_t @ W_out.T + b_out

Key algebraic fusion: the cell input is the previous hidden state, so
    gates = h @ (W_ih + W_hh).T + (b_ih + b_hh)
one matmul per step instead of two.

Distribution: pure data-parallel over the batch dim, 256 rows per core on
8 NeuronCores, weights replicated, no collectives.

On-chip layout is "H-major": the hidden state lives transposed as
hT[hid=1024 part-dim over 8 tiles of 128, batch=256 free-dim].  The
per-step matmul is then gatesT[g,b] = sum_k WT[k,g] * hT[k,b] with the
(fused, transposed) weight as the stationary operand and hT as the moving
operand -- the recurrence needs no transposes anywhere, and the gate bias
is constant along the free dim, i.e. a per-partition scalar that the
scalar engine's activation instruction adds for free.

Matmul dtype: float32r (fp32 stored in SBUF, PE truncates to FP22 -- 1.5
cycles/row vs 2.0 for exact fp32, ~6e-5 per-element rounding).
"""

import os
import sys

sys.path.insert(0, "/opt/trn_rl_repo")

import numpy as np

B, IN_DIM, HID, T, OUT_DIM = 2048, 1024, 1024, 32, 512
N_CORES = 8
BS = B // N_CORES  # batch rows per core
KT = HID // 128  # contraction tiles (8)
GT = 4 * HID // 128  # gate tiles (32)
MT_OUT = OUT_DIM // 128  # out-proj tiles (4)

# "fp16" (default) | "bf16" | "fp32r"
MM_MODE = os.environ.get("BASS_LSTM_MMDT", "fp16")

_cache = {}


def _np_w_dtype():
    if MM_MODE == "bf16":
        import ml_dtypes

        return ml_dtypes.bfloat16
    if MM_MODE == "fp16":
        return np.float16
    return np.float32


def _build():
    import concourse.bacc as bacc
    import concourse.mybir as mybir
    from concourse.tile import TileContext

    f32 = mybir.dt.float32
    if MM_MODE == "bf16":
        dt_w = mybir.dt.bfloat16
    elif MM_MODE == "fp16":
        dt_w = mybir.dt.float16
    else:
        dt_w = f32
    AF = mybir.ActivationFunctionType

    def mm_ap(ap):
        # fp32r is layout-identical to fp32; flip the dtype right at the
        # matmul operand so PE runs the 1.5-cycle FP22 path.
        if MM_MODE == "fp32r":
            return ap.bitcast(mybir.dt.float32r)
        return ap

    nc = bacc.Bacc(
        "TRN2", target_bir_lowering=False, debug=False, num_devices=N_CORES
    )

    xT = nc.dram_tensor("xT", [IN_DIM, BS], dt_w, kind="ExternalInput")
    wg = nc.dram_tensor("wg", [HID, 4 * HID], dt_w, kind="ExternalInput")
    wi = nc.dram_tensor("wi", [IN_DIM, HID], dt_w, kind="ExternalInput")
    wo = nc.dram_tensor("wo", [HID, OUT_DIM], dt_w, kind="ExternalInput")
    bg = nc.dram_tensor("bg", [128, GT], f32, kind="ExternalInput")
    bi = nc.dram_tensor("bi", [128, KT], f32, kind="ExternalInput")
    bo = nc.dram_tensor("bo", [128, MT_OUT], f32, kind="ExternalInput")
    outT = nc.dram_tensor("outT", [T, OUT_DIM, BS], f32, kind="ExternalOutput")

    with TileContext(nc) as tc:
        with (
            tc.tile_pool(name="wgp", bufs=1) as wg_pool,
            tc.tile_pool(name="wop", bufs=1) as wo_pool,
            tc.tile_pool(name="state", bufs=1) as st_pool,
            tc.tile_pool(name="biasp", bufs=1) as b_pool,
            tc.tile_pool(name="gpsum", bufs=6, space="PSUM") as gp_pool,
            tc.tile_pool(name="opsum", bufs=2, space="PSUM") as op_pool,
        ):
            # tiny bias loads first (first ACT needs bi almost immediately)
            bg_sb = b_pool.tile([128, GT], f32, tag="bg", name="bg_sb")
            nc.sync.dma_start(out=bg_sb[:], in_=bg[:, :])
            bi_sb = b_pool.tile([128, KT], f32, tag="bi", name="bi_sb")
            nc.sync.dma_start(out=bi_sb[:], in_=bi[:, :])
            bo_sb = b_pool.tile([128, MT_OUT], f32, tag="bo", name="bo_sb")
            nc.sync.dma_start(out=bo_sb[:], in_=bo[:, :])

            # double-buffered hidden state (H-major), persistent cell state
            h_tiles = [
                [
                    st_pool.tile([128, BS], dt_w, tag=f"h{p}_{j}", name=f"h{p}_{j}")
                    for j in range(KT)
                ]
                for p in range(2)
            ]
            c_tiles = [
                st_pool.tile([128, BS], f32, tag=f"c{j}", name=f"c{j}")
                for j in range(KT)
            ]
            for j in range(KT):
                nc.vector.memset(c_tiles[j][:], 0.0)

            # ---- input projection: h0 = relu(W_in @ x.T + b_in), H-major
            # DMA plan (step 0 needs ALL of wg; h0 needs x + w_in):
            #   sync   : x, w_in (gates h0), then wg[6], wg[7]
            #   gpsimd : wg[0..3]          (~187 GB/s measured)
            #   scalar : wg[4], wg[5], wout (~117 GB/s measured)
            with (
                tc.tile_pool(name="xp", bufs=1) as x_pool,
                tc.tile_pool(name="winp", bufs=1) as wi_pool,
            ):
                xT_sb = []
                for k in range(KT):
                    xt = x_pool.tile([128, BS], dt_w, tag=f"x{k}", name=f"x{k}")
                    nc.sync.dma_start(out=xt[:], in_=xT[k * 128 : (k + 1) * 128, :])
                    xT_sb.append(xt)
                wi_sb = []
                for k in range(KT):
                    wt = wi_pool.tile([128, HID], dt_w, tag=f"wi{k}", name=f"wi{k}")
                    nc.sync.dma_start(out=wt[:], in_=wi[k * 128 : (k + 1) * 128, :])
                    wi_sb.append(wt)

                wg_engine = {0: nc.gpsimd, 1: nc.gpsimd, 2: nc.gpsimd,
                             3: nc.gpsimd, 4: nc.scalar, 5: nc.scalar,
                             6: nc.sync, 7: nc.sync}
                wg_sb = []
                for k in range(KT):
                    wt = wg_pool.tile(
                        [128, 4 * HID], dt_w, tag=f"wg{k}", name=f"wg{k}"
                    )
                    eng = wg_engine[k]
                    half = 2 * HID
                    eng.dma_start(
                        out=wt[:, :half], in_=wg[k * 128 : (k + 1) * 128, :half]
                    )
                    eng.dma_start(
                        out=wt[:, half:], in_=wg[k * 128 : (k + 1) * 128, half:]
                    )
                    wg_sb.append(wt)
                wo_sb = []
                for k in range(KT):
                    wt = wo_pool.tile(
                        [128, OUT_DIM], dt_w, tag=f"wo{k}", name=f"wo{k}"
                    )
                    nc.scalar.dma_start(
                        out=wt[:], in_=wo[k * 128 : (k + 1) * 128, :]
                    )
                    wo_sb.append(wt)

                # k-innermost over 4 concurrent psum groups: each matmul only
                # needs (x[k], w_in[k]), so PE streams behind the sync DMAs
                # instead of waiting for the full w_in.
                for mh in range(2):
                    ms = [mh * 4 + i for i in range(4)]
                    pss = {
                        m: gp_pool.tile([128, BS], f32, tag="gp", name=f"ip_ps{m}")
                        for m in ms
                    }
                    for k in range(KT):
                        for m in ms:
                            nc.tensor.matmul(
                                pss[m][:],
                                lhsT=mm_ap(
                                    wi_sb[k][:, m * 128 : (m + 1) * 128]
                                ),
                                rhs=mm_ap(xT_sb[k][:]),
                                start=(k == 0),
                                stop=(k == KT - 1),
                            )
                    for m in ms:
                        nc.scalar.activation(
                            h_tiles[0][m][:],
                            pss[m][:],
                            AF.Relu,
                            bias=bi_sb[:, m : m + 1],
                        )

            # ---- recurrence
            GATE_FUNCS = [AF.Sigmoid, AF.Sigmoid, AF.Tanh, AF.Sigmoid]  # i f g o
            with (
                tc.tile_pool(name="gact", bufs=12) as ga_pool,
                tc.tile_pool(name="osb", bufs=4) as os_pool,
            ):

                def out_proj(hsrc, t_idx):
                    # outT[t_idx] = W_out @ h + b_out  (H-major h, [OUT,BS] out)
                    for m in range(MT_OUT):
                        ps = op_pool.tile([128, BS], f32, tag="op", name=f"op{t_idx}_{m}")
                        for k in range(KT):
                            nc.tensor.matmul(
                                ps[:],
                                lhsT=mm_ap(wo_sb[k][:, m * 128 : (m + 1) * 128]),
                                rhs=mm_ap(hsrc[k][:]),
                                start=(k == 0),
                                stop=(k == KT - 1),
                            )
                        ot = os_pool.tile([128, BS], f32, tag="ot", name=f"ot{t_idx}_{m}")
                        nc.scalar.activation(
                            ot[:], ps[:], AF.Identity, bias=bo_sb[:, m : m + 1]
                        )
                        nc.sync.dma_start(
                            out=outT[t_idx, m * 128 : (m + 1) * 128, :], in_=ot[:]
                        )

                for t in range(T):
                    hp = h_tiles[t % 2]
                    hn = h_tiles[(t + 1) % 2]
                    for j in range(KT):
                        acts = []
                        for gi in range(4):
                            g_idx = gi * KT + j
                            ps = gp_pool.tile(
                                [128, BS], f32, tag="gp", name=f"ps{t}_{j}_{gi}"
                            )
                            for k in range(KT):
                                nc.tensor.matmul(
                                    ps[:],
                                    lhsT=mm_ap(
                                        wg_sb[k][:, g_idx * 128 : (g_idx + 1) * 128]
                                    ),
                                    rhs=mm_ap(hp[k][:]),
                                    start=(k == 0),
                                    stop=(k == KT - 1),
                                )
                            at = ga_pool.tile(
                                [128, BS], f32, tag="ga", name=f"a{t}_{j}_{gi}"
                            )
                            nc.scalar.activation(
                                at[:],
                                ps[:],
                                GATE_FUNCS[gi],
                                bias=bg_sb[:, g_idx : g_idx + 1],
                            )
                            acts.append(at)
                        a_i, a_f, a_g, a_o = acts
                        fc = ga_pool.tile([128, BS], f32, tag="ga", name=f"fc{t}_{j}")
                        nc.vector.tensor_mul(fc[:], a_f[:], c_tiles[j][:])
                        ig = ga_pool.tile([128, BS], f32, tag="ga", name=f"ig{t}_{j}")
                        nc.vector.tensor_mul(ig[:], a_i[:], a_g[:])
                        nc.vector.tensor_add(c_tiles[j][:], fc[:], ig[:])
                        th = ga_pool.tile([128, BS], f32, tag="ga", name=f"th{t}_{j}")
                        nc.scalar.activation(th[:], c_tiles[j][:], AF.Tanh)
                        nc.vector.tensor_mul(hn[j][:], a_o[:], th[:])
                    if t >= 1:
                        # project the PREVIOUS step's h: its inputs are ready,
                        # so these matmuls keep PE busy while the ACT/DVE
                        # epilogue of this step finishes h_{t+1}.
                        out_proj(hp, t - 1)
                out_proj(h_tiles[T % 2], T - 1)

    nc.finalize()
    return nc


def _get_nc():
    if "nc" not in _cache:
        _cache["nc"] = _build()
    return _cache["nc"]


def _prep_in_maps(x, W_in, b_in, W_ih, b_ih, W_hh, b_hh, W_out, b_out):
    wdt = _np_w_dtype()

    f32 = np.float32
    wg_np = np.ascontiguousarray((W_ih + W_hh).T.astype(wdt))
    wi_np = np.ascontiguousarray(W_in.T.astype(wdt))
    wo_np = np.ascontiguousarray(W_out.T.astype(wdt))
    bg_np = np.ascontiguousarray((b_ih + b_hh).astype(f32).reshape(GT, 128).T)
    bi_np = np.ascontiguousarray(b_in.astype(f32).reshape(KT, 128).T)
    bo_np = np.ascontiguousarray(b_out.astype(f32).reshape(MT_OUT, 128).T)

    in_maps = []
    for c in range(N_CORES):
        xT_c = np.ascontiguousarray(x[c * BS : (c + 1) * BS].T.astype(wdt))
        in_maps.append(
            {
                "xT": xT_c,
                "wg": wg_np,
                "wi": wi_np,
                "wo": wo_np,
                "bg": bg_np,
                "bi": bi_np,
                "bo": bo_np,
            }
        )

    return in_maps


def _assemble(results):
    out = np.concatenate(
        [results[c]["outT"].transpose(2, 0, 1) for c in range(N_CORES)], axis=0
    )
    return np.ascontiguousarray(out.astype(np.float32))


def kernel(x, W_in, b_in, W_ih, b_ih, W_hh, b_hh, W_out, b_out):
    from concourse.bass_utils import run_bass_kernel_spmd

    nc = _get_nc()
    in_maps = _prep_in_maps(
        x, W_in, b_in, W_ih, b_ih, W_hh, b_hh, W_out, b_out
    )
    res = run_bass_kernel_spmd(nc, in_maps, list(range(N_CORES)))
    return _assemble(res.results)
